# revision 16
# baseline (speedup 1.0000x reference)
"""PointNetLK forward on 8 Trainium2 NeuronCores (Bass/Tile), pure data parallel.

Contract: kernel(**inputs) takes the FULL unsharded inputs from
reference.setup_inputs() and returns the FULL [B, N, 3] output.

Per-core program (B=8 sharded one sample per core):
  - mean-center template/source (feature-major [3, N] layout)
  - f0 = PointNet feature of centered template (5-layer MLP + max-pool)
  - chamfer nearest-source distance per template point via one augmented
    matmul (-d2 = 2 t.s - |t|^2 - |s|^2), max-reduce -> -mind
  - exact top-512 selection: rank_i = #{v_j > v_i} + #{j<i: v_j == v_i}
    (tie-break identical to jax.lax.top_k), keypoints gathered with a
    one-hot selection matmul
  - finite-difference Jacobian over the 6 se3 twists (twist transforms
    folded into layer-1 weights; twists are compile-time constants)
  - H = J^T J, Hinv by Gauss-Jordan (no pivoting; H is SPD)
  - maxiter LK iterations: r = f(T(src)) - f0, pose = -Hinv (J^T r),
    se3_exp via even polynomials in th^2 (no ACT table switches),
    compose T.  (Reference's global convergence check never fires at
    these tolerances; increments are ~1e-2..1e-3 >> xtol=1e-7.)
  - output y = R src + t, written feature-major, transposed on host.
"""

import math
import sys

for _p in ("/opt/trn_rl_repo",):
    if _p not in sys.path:
        sys.path.insert(0, _p)

import numpy as np

import concourse.bass as bass
import concourse.mybir as mybir
import concourse.tile as tile
from concourse.alu_op_type import AluOpType
from concourse.mybir import ActivationFunctionType as AF
from concourse.mybir import AxisListType

F32 = mybir.dt.float32
DELTA = 0.01
K_KEY = 512
N_PTS = 2048
N_CHUNK = N_PTS // 512  # 4
DIMS = [3, 64, 64, 64, 128, 1024]
DEG = 6  # Horner degree for the even se3 coefficient polynomials


# ---------------------------------------------------------------- host math
def _se3_exp_host(xi):
    """fp32 se3 exp for the 6 constant twists (mirrors reference numerics)."""
    xi = xi.astype(np.float32)
    w, v = xi[:3], xi[3:]
    th = np.float32(np.sqrt(np.float32((w * w).sum())))
    wx = np.array(
        [[0, -w[2], w[1]], [w[2], 0, -w[0]], [-w[1], w[0], 0]], np.float32
    )
    wx2 = (wx @ wx).astype(np.float32)
    safe = max(th, np.float32(1e-8))
    if th < 1e-4:
        A = 1.0 - th * th / 6.0
        B = 0.5 - th * th / 24.0
        C = 1.0 / 6.0 - th * th / 120.0
    else:
        A = np.sin(safe) / safe
        B = (1.0 - np.cos(safe)) / (safe * safe)
        C = (safe - np.sin(safe)) / safe**3
    A, B, C = np.float32(A), np.float32(B), np.float32(C)
    I = np.eye(3, dtype=np.float32)
    R = (I + A * wx + B * wx2).astype(np.float32)
    V = (I + B * wx + C * wx2).astype(np.float32)
    t = (V @ v).astype(np.float32)
    return R, t


def _coef_table():
    """[4, DEG+1] low-order-first Taylor coeffs of A,B,C,D as functions of x=th^2."""
    A = [(-1) ** k / math.factorial(2 * k + 1) for k in range(DEG + 1)]
    B = [(-1) ** k / math.factorial(2 * k + 2) for k in range(DEG + 1)]
    C = [(-1) ** k / math.factorial(2 * k + 3) for k in range(DEG + 1)]
    D = [(-1) ** k / math.factorial(2 * k) for k in range(DEG + 1)]
    tbl = np.array([A, B, C, D], np.float32)  # [4, DEG+1]
    return np.ascontiguousarray(tbl.T.reshape(1, -1))  # [1, 4*(DEG+1)], degree-major


def host_constants():
    c = {}
    jrot = np.zeros((3, 18), np.float32)
    jtr = np.zeros((3, 6), np.float32)
    for p in range(6):
        xi = np.zeros(6, np.float32)
        xi[p] = np.float32(-DELTA)
        R, t = _se3_exp_host(xi)
        jrot[:, 3 * p : 3 * p + 3] = R
        jtr[:, p] = t
    c["jrot"] = jrot
    c["jtr"] = jtr
    c["coef"] = _coef_table()
    c["iota512"] = np.broadcast_to(
        np.arange(512, dtype=np.float32)[None, :], (128, 512)
    ).copy()
    c["tri128"] = (
        np.arange(128)[None, :] < np.arange(128)[:, None]
    ).astype(np.float32)
    c["ident"] = np.eye(128, dtype=np.float32)
    c["rowbase"] = (
        np.arange(128, dtype=np.float32)[:, None]
        + 128.0 * np.arange(16, dtype=np.float32)[None, :]
    ).astype(np.float32)
    # rt12 [1,12] -> Rt [3,4] rebuild helpers: mod3 selector and div3 mask
    mod3 = np.zeros((12, 3), np.float32)
    maskc = np.zeros((12, 4), np.float32)
    for j in range(12):
        mod3[j, j % 3] = 1.0
        maskc[j, j // 3] = 1.0
    c["rtc"] = np.concatenate([mod3, maskc], axis=1).astype(np.float32)
    # Gauss-Jordan helpers: ek6 row-selector blocks, 1-e_k masks, ident6
    ek6 = np.zeros((6, 36), np.float32)
    for k in range(6):
        ek6[k, 6 * k : 6 * k + 6] = 1.0
    gjmask = np.ones((6, 6), np.float32) - np.eye(6, dtype=np.float32)
    c["gjc"] = np.concatenate([ek6, gjmask, np.eye(6, dtype=np.float32)],
                              axis=1).astype(np.float32)
    # i9 / sgn9 packed [1, 18]
    i9 = np.array([1, 0, 0, 0, 1, 0, 0, 0, 1], np.float32)
    sgn9 = np.array([0, -1, 1, 1, 0, -1, -1, 1, 0], np.float32)
    c["i9sgn"] = np.concatenate([i9, sgn9])[None, :].astype(np.float32)
    return c


# ------------------------------------------------------------- program build
def split_multiwait_drains(nc, max_waits=1):
    """This image's walrus rejects >1 sync-wait command per instruction
    (CTRL Drains and Matmult/LDWEIGHTS alike).  Hoist all but the last wait
    of any multi-wait instruction onto single-wait Nops inserted just before
    it on the same engine."""
    n = 0
    for fn in nc.m.functions:
        for b in fn.blocks:
            il = b.instructions
            i = 0
            while i < len(il):
                inst = il[i]
                si = inst.sync_info
                if si is not None and si.on_wait and len(si.on_wait) > max_waits:
                    waits = list(si.on_wait)
                    for j, w in enumerate(waits[:-max_waits]):
                        d = mybir.InstNoOp(
                            name=f"{inst.name}-sw{j}", ins=[], outs=[]
                        )
                        d.engine = inst.engine
                        d.sync_info = mybir.SyncInfo(on_wait=[w], on_update=[])
                        il.insert(i, d)
                        i += 1
                        n += 1
                    si.on_wait = waits[-max_waits:]
                i += 1
    return n


def build_program(maxiter, debug=False):
    nc = bass.Bass("TRN2", debug=False)

    # ---------------- DRAM I/O ----------------
    d_tT = nc.dram_tensor("tT", [3, N_PTS], F32, kind="ExternalInput")
    d_sT = nc.dram_tensor("sT", [3, N_PTS], F32, kind="ExternalInput")
    d_W = [
        nc.dram_tensor(f"W{i+1}", [DIMS[i], DIMS[i + 1]], F32, kind="ExternalInput")
        for i in range(5)
    ]
    d_b = [
        nc.dram_tensor(f"b{i+1}", [DIMS[i + 1], 1], F32, kind="ExternalInput")
        for i in range(4)
    ]
    d_b5 = nc.dram_tensor("b5", [128, 8], F32, kind="ExternalInput")
    d_jrot = nc.dram_tensor("jrot", [3, 18], F32, kind="ExternalInput")
    d_jtr = nc.dram_tensor("jtr", [3, 6], F32, kind="ExternalInput")
    d_coef = nc.dram_tensor("coef", [1, 4 * (DEG + 1)], F32, kind="ExternalInput")
    d_iota = nc.dram_tensor("iota512", [128, 512], F32, kind="ExternalInput")
    d_tri = nc.dram_tensor("tri128", [128, 128], F32, kind="ExternalInput")
    d_ident = nc.dram_tensor("ident", [128, 128], F32, kind="ExternalInput")
    d_rowb = nc.dram_tensor("rowbase", [128, 16], F32, kind="ExternalInput")
    d_i9 = nc.dram_tensor("i9sgn", [1, 18], F32, kind="ExternalInput")
    d_gjc = nc.dram_tensor("gjc", [6, 48], F32, kind="ExternalInput")
    d_rtc = nc.dram_tensor("rtc", [12, 7], F32, kind="ExternalInput")
    d_out = nc.dram_tensor("yT", [3, N_PTS], F32, kind="ExternalOutput")
    d_dbg = None
    if debug:
        d_dbg = {
            "dbg_f0": nc.dram_tensor("dbg_f0", [128, 8], F32, kind="ExternalOutput"),
            "dbg_negmind": nc.dram_tensor("dbg_negmind", [128, 16], F32, kind="ExternalOutput"),
            "dbg_rank": nc.dram_tensor("dbg_rank", [128, 16], F32, kind="ExternalOutput"),
            "dbg_kpT": nc.dram_tensor("dbg_kpT", [3, 512], F32, kind="ExternalOutput"),
            "dbg_J": nc.dram_tensor("dbg_J", [128, 48], F32, kind="ExternalOutput"),
            "dbg_M": nc.dram_tensor("dbg_M", [6, 12], F32, kind="ExternalOutput"),
            "dbg_pose": nc.dram_tensor("dbg_pose", [6, 1], F32, kind="ExternalOutput"),
            "dbg_rt12": nc.dram_tensor("dbg_rt12", [1, 12], F32, kind="ExternalOutput"),
            "dbg_Rt": nc.dram_tensor("dbg_Rt", [3, 4], F32, kind="ExternalOutput"),
            "dbg_Rcur1": nc.dram_tensor("dbg_Rcur1", [3, 3], F32, kind="ExternalOutput"),
        }

    with tile.TileContext(nc) as tc:
        _build_body(
            nc, tc, maxiter,
            d_tT, d_sT, d_W, d_b, d_b5, d_jrot, d_jtr, d_coef,
            d_iota, d_tri, d_ident, d_rowb, d_i9, d_gjc, d_rtc, d_out, d_dbg,
        )
    return nc


def _build_body(
    nc, tc, maxiter,
    d_tT, d_sT, d_W, d_b, d_b5, d_jrot, d_jtr, d_coef,
    d_iota, d_tri, d_ident, d_rowb, d_i9, d_gjc, d_rtc, d_out, d_dbg=None,
):
    from contextlib import ExitStack

    ctx = ExitStack()
    persist = ctx.enter_context(tc.tile_pool(name="persist", bufs=1))
    acts = ctx.enter_context(tc.tile_pool(name="acts", bufs=1))
    work = ctx.enter_context(tc.tile_pool(name="work", bufs=2))
    setup = ctx.enter_context(tc.tile_pool(name="setup", bufs=1))
    small = ctx.enter_context(tc.tile_pool(name="small", bufs=2))
    pp = ctx.enter_context(tc.tile_pool(name="pp", bufs=4, space="PSUM"))
    pps = ctx.enter_context(tc.tile_pool(name="pps", bufs=2, space="PSUM"))
    ppacc = ctx.enter_context(tc.tile_pool(name="ppacc", bufs=1, space="PSUM"))

    dma = nc.sync.dma_start
    act = nc.scalar.activation
    V = nc.vector

    # ---------------- load inputs / constants ----------------
    tT = persist.tile([3, N_PTS], F32, tag="tT")
    sT = persist.tile([3, N_PTS], F32, tag="sT")
    dma(tT[:, :], d_tT.ap())
    dma(sT[:, :], d_sT.ap())
    W = []
    for i in range(5):
        w = persist.tile([DIMS[i], DIMS[i + 1]], F32, tag=f"W{i+1}")
        dma(w[:, :], d_W[i].ap())
        W.append(w)
    b = []
    for i in range(4):
        t = persist.tile([DIMS[i + 1], 1], F32, tag=f"b{i+1}")
        dma(t[:, :], d_b[i].ap())
        b.append(t)
    b5 = persist.tile([128, 8], F32, tag="b5")
    dma(b5[:, :], d_b5.ap())
    jrot = persist.tile([3, 18], F32, tag="jrot")
    dma(jrot[:, :], d_jrot.ap())
    jtr = persist.tile([3, 6], F32, tag="jtr")
    dma(jtr[:, :], d_jtr.ap())
    coef = persist.tile([1, 4 * (DEG + 1)], F32, tag="coef")
    dma(coef[:, :], d_coef.ap())
    iota512 = persist.tile([128, 512], F32, tag="iota512")
    dma(iota512[:, :], d_iota.ap())
    tri = persist.tile([128, 128], F32, tag="tri")
    dma(tri[:, :], d_tri.ap())
    ident = persist.tile([128, 128], F32, tag="ident")
    dma(ident[:, :], d_ident.ap())
    rowb = persist.tile([128, 16], F32, tag="rowb")
    dma(rowb[:, :], d_rowb.ap())
    i9sgn = persist.tile([1, 18], F32, tag="i9sgn")
    dma(i9sgn[:, :], d_i9.ap())
    gjc = persist.tile([6, 48], F32, tag="gjc")
    dma(gjc[:, :], d_gjc.ap())
    rtc = persist.tile([12, 7], F32, tag="rtc")
    dma(rtc[:, :], d_rtc.ap())
    i9 = i9sgn[:, 0:9]
    sgn9 = i9sgn[:, 9:18]

    ones3 = persist.tile([3, 1], F32, tag="ones3")
    V.memset(ones3[:, :], 1.0)
    ones1w = persist.tile([1, 128], F32, tag="ones1w")
    V.memset(ones1w[:, :], 1.0)

    # ---------------- mean-center ----------------
    tm = persist.tile([3, 1], F32, tag="tm")
    sm = persist.tile([3, 1], F32, tag="sm")
    tmp31 = small.tile([3, 1], F32, tag="tmp31")
    V.reduce_sum(tmp31[:, :], tT[:, :], axis=AxisListType.X)
    V.tensor_scalar(tm[:, :], tmp31[:, :], 1.0 / N_PTS, None, AluOpType.mult)
    tmp31b = small.tile([3, 1], F32, tag="tmp31")
    V.reduce_sum(tmp31b[:, :], sT[:, :], axis=AxisListType.X)
    V.tensor_scalar(sm[:, :], tmp31b[:, :], 1.0 / N_PTS, None, AluOpType.mult)

    tcT = persist.tile([3, N_PTS], F32, tag="tcT")
    scT = persist.tile([3, N_PTS], F32, tag="scT")
    V.tensor_scalar(tcT[:, :], tT[:, :], tm[:, :], None, AluOpType.subtract)
    V.tensor_scalar(scT[:, :], sT[:, :], sm[:, :], None, AluOpType.subtract)

    # ---------------- shared MLP emitter ----------------
    def emit_mlp(src_of_chunk, n_chunks, w1_list, b1_list, fraw, col_of_chunk):
        """src_of_chunk(j) -> [3,512] sbuf slice; w1_list/b1_list: per-chunk
        (or len-1) layer-1 weight [3,64] / bias [64,1] tiles; fraw: [128,
        ncols] output, raw (pre-bias, pre-relu) per-block maxes;
        col_of_chunk(c, j) gives the fraw column for block c, chunk j.
        Caller applies bias+relu."""
        a1 = acts.tile([64, 512 * n_chunks], F32, tag="a1")
        a2 = acts.tile([64, 512 * n_chunks], F32, tag="a2")
        a3 = acts.tile([64, 512 * n_chunks], F32, tag="a3")
        a4 = acts.tile([128, 512 * n_chunks], F32, tag="a4")
        for j in range(n_chunks):
            w1 = w1_list[j if len(w1_list) > 1 else 0]
            b1x = b1_list[j if len(b1_list) > 1 else 0]
            ps = pp.tile([128, 512], F32, tag="mm")
            nc.tensor.matmul(ps[0:64, :], w1[:, :], src_of_chunk(j),
                             start=True, stop=True)
            act(a1[:, 512 * j : 512 * (j + 1)], ps[0:64, :], AF.Relu, bias=b1x[:, :])
        for (win, bin_, ain, aout) in ((W[1], b[1], a1, a2), (W[2], b[2], a2, a3)):
            for j in range(n_chunks):
                ps = pp.tile([128, 512], F32, tag="mm")
                nc.tensor.matmul(ps[0:64, :], win[:, :],
                                 ain[:, 512 * j : 512 * (j + 1)], start=True, stop=True)
                act(aout[:, 512 * j : 512 * (j + 1)], ps[0:64, :], AF.Relu, bias=bin_[:, :])
        for j in range(n_chunks):
            ps = pp.tile([128, 512], F32, tag="mm")
            nc.tensor.matmul(ps[:, :], W[3][:, :], a3[:, 512 * j : 512 * (j + 1)],
                             start=True, stop=True)
            act(a4[:, 512 * j : 512 * (j + 1)], ps[:, :], AF.Relu, bias=b[3][:, :])
        for c in range(8):
            for j in range(n_chunks):
                ps = pp.tile([128, 512], F32, tag="mm")
                nc.tensor.matmul(ps[:, :], W[4][:, 128 * c : 128 * (c + 1)],
                                 a4[:, 512 * j : 512 * (j + 1)], start=True, stop=True)
                col = col_of_chunk(c, j)
                V.reduce_max(fraw[:, col : col + 1], ps[:, :], axis=AxisListType.X)

    # ---------------- f0 ----------------
    f0 = persist.tile([128, 8], F32, tag="f0")
    f0raw = work.tile([128, 32], F32, tag="f0raw")
    emit_mlp(lambda j: tcT[:, 512 * j : 512 * (j + 1)], N_CHUNK,
             [W[0]], [b[0]], f0raw, lambda c, j: 4 * c + j)
    f0m = work.tile([128, 8], F32, tag="f0m")
    for c in range(8):
        V.reduce_max(f0m[:, c : c + 1], f0raw[:, 4 * c : 4 * (c + 1)],
                     axis=AxisListType.X)
    V.tensor_add(f0[:, :], f0m[:, :], b5[:, :])
    V.tensor_scalar(f0[:, :], f0[:, :], 0.0, None, AluOpType.max)

    # ---------------- chamfer: -mind per template point ----------------
    Taug = persist.tile([5, N_PTS], F32, tag="Taug")
    Saug = persist.tile([5, N_PTS], F32, tag="Saug")
    V.tensor_scalar(Taug[0:3, :], tcT[:, :], 2.0, None, AluOpType.mult)
    V.tensor_copy(Saug[0:3, :], scT[:, :])
    ones_row = setup.tile([1, N_PTS], F32, tag="bigB")
    V.memset(ones_row[:, :], 1.0)
    dma(Taug[4:5, :], ones_row[:, :])
    dma(Saug[3:4, :], ones_row[:, :])
    tsq = setup.tile([3, N_PTS], F32, tag="bigA")
    stage = setup.tile([1, N_PTS], F32, tag="bigC")
    V.tensor_mul(tsq[:, :], tcT[:, :], tcT[:, :])
    for j in range(N_CHUNK):
        ps = pps.tile([128, 512], F32, tag="sm")
        nc.tensor.matmul(ps[0:1, 0:512], ones3[:, :], tsq[:, 512 * j : 512 * (j + 1)],
                         start=True, stop=True)
        act(stage[0:1, 512 * j : 512 * (j + 1)], ps[0:1, 0:512], AF.Copy, scale=-1.0)
    dma(Taug[3:4, :], stage[:, :])
    stage2 = setup.tile([1, N_PTS], F32, tag="bigC")
    V.tensor_mul(tsq[:, :], scT[:, :], scT[:, :])
    for j in range(N_CHUNK):
        ps = pps.tile([128, 512], F32, tag="sm")
        nc.tensor.matmul(ps[0:1, 0:512], ones3[:, :], tsq[:, 512 * j : 512 * (j + 1)],
                         start=True, stop=True)
        act(stage2[0:1, 512 * j : 512 * (j + 1)], ps[0:1, 0:512], AF.Copy, scale=-1.0)
    dma(Saug[4:5, :], stage2[:, :])

    negmind = persist.tile([128, 16], F32, tag="negmind")
    for c in range(16):
        m4 = small.tile([128, 4], F32, tag="m4")
        for j in range(N_CHUNK):
            ps = pp.tile([128, 512], F32, tag="mm")
            nc.tensor.matmul(ps[:, :], Taug[:, 128 * c : 128 * (c + 1)],
                             Saug[:, 512 * j : 512 * (j + 1)], start=True, stop=True)
            V.reduce_max(m4[:, j : j + 1], ps[:, :], axis=AxisListType.X)
        V.reduce_max(negmind[:, c : c + 1], m4[:, :], axis=AxisListType.X)

    # ---------------- exact descending rank with index tie-break ----------
    # nm_row [1, 2048] then broadcast to [128, 2048]
    ps_t = pps.tile([128, 512], F32, tag="sm")
    nc.tensor.transpose(ps_t[0:16, 0:128], negmind[:, :], ident[:, :])
    nm_t = work.tile([16, 128], F32, tag="nm_t")
    act(nm_t[:, :], ps_t[0:16, 0:128], AF.Copy)
    nm_row = setup.tile([1, N_PTS], F32, tag="bigB")
    dma(nm_row[0:1, :].rearrange("p (c q) -> p c q", c=16), nm_t[:, :])
    nm_b = persist.tile([128, N_PTS], F32, tag="nm_b")
    for j in range(N_CHUNK):
        ps = pp.tile([128, 512], F32, tag="mm")
        nc.tensor.matmul(ps[:, :], ones1w[:, :], nm_row[:, 512 * j : 512 * (j + 1)],
                         start=True, stop=True)
        act(nm_b[:, 512 * j : 512 * (j + 1)], ps[:, :], AF.Copy)

    rank = persist.tile([128, 16], F32, tag="rank")
    scratch = setup.tile([128, N_PTS], F32, tag="bigA")
    gcnt = work.tile([128, 16], F32, tag="gcnt")
    tcnt = work.tile([128, 16], F32, tag="tcnt")
    for c in range(16):
        V.tensor_scalar(scratch[:, :], nm_b[:, :], negmind[:, c : c + 1], None,
                        AluOpType.is_gt, op1=AluOpType.add,
                        accum_out=gcnt[:, c : c + 1])
    # T2: same-chunk lower-index ties
    for c in range(16):
        V.scalar_tensor_tensor(
            scratch[:, 0:128], nm_b[:, 128 * c : 128 * (c + 1)],
            negmind[:, c : c + 1], tri[:, :],
            AluOpType.is_equal, AluOpType.mult, accum_out=tcnt[:, c : c + 1])
    V.tensor_add(rank[:, :], gcnt[:, :], tcnt[:, :])
    # T1: earlier-chunk ties
    for c in range(1, 16):
        V.tensor_scalar(scratch[:, 0 : 128 * c], nm_b[:, 0 : 128 * c],
                        negmind[:, c : c + 1], None,
                        AluOpType.is_equal, op1=AluOpType.add,
                        accum_out=tcnt[:, c : c + 1])
    V.tensor_add(rank[:, 1:16], rank[:, 1:16], tcnt[:, 1:16])

    # ---------------- gather top-512 template keypoints -------------------
    # template points chunk-transposed: tpts [128, 48], cols 3c:3c+3 = chunk c
    tpts = persist.tile([128, 48], F32, tag="tpts")
    for c in range(16):
        ps = pps.tile([128, 512], F32, tag="sm")
        nc.tensor.transpose(ps[0:128, 0:3], tcT[:, 128 * c : 128 * (c + 1)], ident[0:3, 0:3])
        act(tpts[:, 3 * c : 3 * (c + 1)], ps[0:128, 0:3], AF.Copy)
    kp_ps = ppacc.tile([128, 512], F32, tag="acc")
    for c in range(16):
        sel = work.tile([128, 512], F32, tag="sel")
        V.tensor_scalar(sel[:, :], iota512[:, :], rank[:, c : c + 1], None,
                        AluOpType.is_equal)
        nc.tensor.matmul(kp_ps[0:3, 0:512], tpts[:, 3 * c : 3 * (c + 1)], sel[:, :],
                         start=(c == 0), stop=(c == 15))
    kpT = persist.tile([3, 512], F32, tag="kpT")
    act(kpT[:, :], kp_ps[0:3, 0:512], AF.Copy)

    # ---------------- Jacobian ----------------
    w1e_j = persist.tile([3, 6 * 64], F32, tag="w1e_j")
    b1e_j = persist.tile([64, 6], F32, tag="b1e_j")
    for p in range(6):
        ps = pps.tile([128, 512], F32, tag="sm")
        nc.tensor.matmul(ps[0:3, 0:64], jrot[:, 3 * p : 3 * (p + 1)], W[0][:, :],
                         start=True, stop=True)
        act(w1e_j[:, 64 * p : 64 * (p + 1)], ps[0:3, 0:64], AF.Copy)
        ps2 = pps.tile([128, 512], F32, tag="sm")
        nc.tensor.matmul(ps2[0:64, 0:1], W[0][:, :], jtr[:, p : p + 1],
                         start=True, stop=True)
        V.tensor_scalar(b1e_j[:, p : p + 1], ps2[0:64, 0:1], b[0][:, :], None,
                        AluOpType.add)

    fpraw = work.tile([128, 48], F32, tag="fpraw")
    emit_mlp(
        lambda j: kpT[:, :], 6,
        [w1e_j[:, 64 * p : 64 * (p + 1)] for p in range(6)],
        [b1e_j[:, p : p + 1] for p in range(6)],
        fpraw, lambda c, j: 6 * c + j,
    )
    # fpert = relu(fpraw + b5), J = (f0 - fpert) * 100
    J = persist.tile([128, 48], F32, tag="J")
    fp3 = fpraw[:, :].rearrange("a (c p) -> a c p", p=6)
    V.tensor_add(fp3, fp3, b5[:, :].unsqueeze(2).broadcast_to([128, 8, 6]))
    V.tensor_scalar(fpraw[:, :], fpraw[:, :], 0.0, None, AluOpType.max)
    V.tensor_sub(J[:, :].rearrange("a (c p) -> a c p", p=6),
                 f0[:, :].unsqueeze(2).broadcast_to([128, 8, 6]), fp3)
    V.tensor_scalar(J[:, :], J[:, :], 1.0 / DELTA, None, AluOpType.mult)

    # ---------------- H = J^T J, Hinv by Gauss-Jordan ----------------
    h_ps = ppacc.tile([128, 512], F32, tag="acc")
    for c in range(8):
        nc.tensor.matmul(h_ps[0:6, 0:6], J[:, 6 * c : 6 * (c + 1)],
                         J[:, 6 * c : 6 * (c + 1)], start=(c == 0), stop=(c == 7))
    M = persist.tile([6, 12], F32, tag="gjM")
    act(M[:, 0:6], h_ps[0:6, 0:6], AF.Copy)
    V.tensor_copy(M[:, 6:12], gjc[:, 42:48])
    ek6 = gjc[:, 0:36]
    gjmask = gjc[:, 36:42]
    id6 = gjc[:, 42:48]
    for k in range(6):
        # pivb = ones6 (x) row_k(M): selector matmul, all operands base-0
        pivb_ps = pps.tile([128, 512], F32, tag="sm")
        nc.tensor.matmul(pivb_ps[0:6, 0:12], ek6[:, 6 * k : 6 * k + 6], M[:, :],
                         start=True, stop=True)
        pivb = small.tile([6, 12], F32, tag="pivb")
        act(pivb[:, :], pivb_ps[0:6, 0:12], AF.Copy)
        d = pivb[:, k : k + 1]
        r0 = small.tile([6, 1], F32, tag="gj_r")
        V.reciprocal(r0[:, :], d)
        for _ in range(2):  # Newton polish
            t1 = small.tile([6, 1], F32, tag="gj_t")
            V.tensor_mul(t1[:, :], d, r0[:, :])
            V.tensor_scalar(t1[:, :], t1[:, :], -1.0, 2.0, AluOpType.mult,
                            op1=AluOpType.add)
            V.tensor_mul(r0[:, :], r0[:, :], t1[:, :])
        # v = (M[:,k]*r) . (1-e_k)  +  e_k*(1-r);  M <- M - v (x) row_k(M)
        ta = small.tile([6, 1], F32, tag="gj_ta")
        V.tensor_mul(ta[:, :], M[:, k : k + 1], r0[:, :])
        V.tensor_mul(ta[:, :], ta[:, :], gjmask[:, k : k + 1])
        tb = small.tile([6, 1], F32, tag="gj_tb")
        V.tensor_scalar(tb[:, :], r0[:, :], -1.0, 1.0, AluOpType.mult,
                        op1=AluOpType.add)
        V.tensor_mul(tb[:, :], tb[:, :], id6[:, k : k + 1])
        V.tensor_add(ta[:, :], ta[:, :], tb[:, :])
        rank1 = small.tile([6, 12], F32, tag="rank1")
        V.tensor_scalar(rank1[:, :], pivb[:, :], ta[:, :], None, AluOpType.mult)
        V.tensor_sub(M[:, :], M[:, :], rank1[:, :])
    # pose matmul needs lhsT = Hinv^T so that out = Hinv @ u (GJ result is not
    # exactly symmetric, and the solve is cancellation-sensitive)
    hT_ps = pps.tile([128, 512], F32, tag="sm")
    nc.tensor.transpose(hT_ps[0:6, 0:6], M[:, 6:12], ident[0:6, 0:6])
    HinvT = persist.tile([6, 6], F32, tag="HinvT")
    act(HinvT[:, :], hT_ps[0:6, 0:6], AF.Copy)

    # ---------------- LK iterations ----------------
    Rcur = persist.tile([3, 3], F32, tag="Rcur")
    RcurT = persist.tile([3, 3], F32, tag="RcurT")
    tcur = persist.tile([3, 1], F32, tag="tcur")
    V.tensor_copy(Rcur[:, :], gjc[0:3, 42:45])
    V.tensor_copy(RcurT[:, :], gjc[0:3, 42:45])
    V.memset(tcur[:, :], 0.0)

    for it in range(maxiter):
        # layer-1 fold of current transform
        ps = pps.tile([128, 512], F32, tag="sm")
        nc.tensor.matmul(ps[0:3, 0:64], Rcur[:, :], W[0][:, :], start=True, stop=True)
        w1e = work.tile([3, 64], F32, tag="w1e")
        act(w1e[:, :], ps[0:3, 0:64], AF.Copy)
        ps2 = pps.tile([128, 512], F32, tag="sm")
        nc.tensor.matmul(ps2[0:64, 0:1], W[0][:, :], tcur[:, :], start=True, stop=True)
        b1e = work.tile([64, 1], F32, tag="b1e")
        V.tensor_scalar(b1e[:, :], ps2[0:64, 0:1], b[0][:, :], None, AluOpType.add)

        fraw = work.tile([128, 32], F32, tag="f0raw")
        emit_mlp(lambda j: scT[:, 512 * j : 512 * (j + 1)], N_CHUNK,
                 [w1e], [b1e], fraw, lambda c, j: 4 * c + j)
        f = work.tile([128, 8], F32, tag="fcur")
        for c in range(8):
            V.reduce_max(f[:, c : c + 1], fraw[:, 4 * c : 4 * (c + 1)],
                         axis=AxisListType.X)
        V.tensor_add(f[:, :], f[:, :], b5[:, :])
        V.tensor_scalar(f[:, :], f[:, :], 0.0, None, AluOpType.max)
        r = work.tile([128, 8], F32, tag="resid")
        V.tensor_sub(r[:, :], f[:, :], f0[:, :])

        u_ps = ppacc.tile([128, 512], F32, tag="acc")
        for c in range(8):
            nc.tensor.matmul(u_ps[0:6, 0:1], J[:, 6 * c : 6 * (c + 1)],
                             r[:, c : c + 1], start=(c == 0), stop=(c == 7))
        u = small.tile([6, 1], F32, tag="u_sb")
        act(u[:, :], u_ps[0:6, 0:1], AF.Copy)
        pose_ps = pps.tile([128, 512], F32, tag="sm")
        nc.tensor.matmul(pose_ps[0:6, 0:1], HinvT[:, :], u[:, :], start=True, stop=True)
        pose = small.tile([6, 1], F32, tag="pose_sb")
        act(pose[:, :], pose_ps[0:6, 0:1], AF.Copy, scale=-1.0)

        # se3_exp via even polynomials of x = |w|^2
        pt_ps = pps.tile([128, 512], F32, tag="sm")
        nc.tensor.transpose(pt_ps[0:1, 0:6], pose[:, :], ident[0:6, 0:6])
        poseT = small.tile([1, 6], F32, tag="poseT_sb")
        act(poseT[:, :], pt_ps[0:1, 0:6], AF.Copy)
        x_ps = pps.tile([128, 512], F32, tag="sm")
        nc.tensor.matmul(x_ps[0:1, 0:1], pose[0:3, :], pose[0:3, :],
                         start=True, stop=True)
        x1 = small.tile([1, 1], F32, tag="x1")
        act(x1[:, :], x_ps[0:1, 0:1], AF.Copy)

        abcd = small.tile([1, 4], F32, tag="abcd")
        V.tensor_copy(abcd[:, :], coef[:, 4 * DEG : 4 * DEG + 4])
        for dgr in range(DEG - 1, -1, -1):
            V.tensor_scalar(abcd[:, :], abcd[:, :], x1[:, :], None, AluOpType.mult)
            V.tensor_add(abcd[:, :], abcd[:, :], coef[:, 4 * dgr : 4 * dgr + 4])
        cA, cB, cC, cD = (abcd[:, 0:1], abcd[:, 1:2],
                          abcd[:, 2:3], abcd[:, 3:4])

        w3 = poseT[:, 0:3]
        v3 = poseT[:, 3:6]
        rt12 = small.tile([1, 12], F32, tag="rt12")
        wrep = w3.unsqueeze(2).broadcast_to([1, 3, 3])      # w0 w0 w0 w1 ...
        wtile = w3.unsqueeze(1).broadcast_to([1, 3, 3])     # w0 w1 w2 w0 ...
        ww = small.tile([1, 9], F32, tag="ww")
        V.tensor_mul(ww[:, :].rearrange("p (a c) -> p a c", a=3), wrep, wtile)
        skew = small.tile([1, 12], F32, tag="skew")
        V.memset(skew[:, :], 0.0)
        # w0 -> slots 5,7 ; w1 -> slots 2,6 ; w2 -> slots 1,3
        V.tensor_copy(skew[:, 5:9].rearrange("p (a c) -> p a c", a=2, c=2)[:, :, 0:1],
                      w3[:, 0:1].unsqueeze(1).broadcast_to([1, 2, 1]))
        V.tensor_copy(skew[:, 2:10].rearrange("p (a c) -> p a c", a=2, c=4)[:, :, 0:1],
                      w3[:, 1:2].unsqueeze(1).broadcast_to([1, 2, 1]))
        V.tensor_copy(skew[:, 1:5].rearrange("p (a c) -> p a c", a=2, c=2)[:, :, 0:1],
                      w3[:, 2:3].unsqueeze(1).broadcast_to([1, 2, 1]))
        V.tensor_mul(skew[:, 0:9], skew[:, 0:9], sgn9)
        # Rvec = D*I9 + A*skew + B*ww  (into rt12[0:9])
        rv = rt12[:, 0:9]
        V.tensor_scalar(rv, i9, cD, None, AluOpType.mult)
        V.scalar_tensor_tensor(rv, skew[:, 0:9], cA, rv, AluOpType.mult, AluOpType.add)
        V.scalar_tensor_tensor(rv, ww[:, :], cB, rv, AluOpType.mult, AluOpType.add)
        # Vvec = A*I9 + B*skew + C*ww
        vv = small.tile([1, 9], F32, tag="vv")
        V.tensor_scalar(vv[:, :], i9, cA, None, AluOpType.mult)
        V.scalar_tensor_tensor(vv[:, :], skew[:, 0:9], cB, vv[:, :],
                               AluOpType.mult, AluOpType.add)
        V.scalar_tensor_tensor(vv[:, :], ww[:, :], cC, vv[:, :],
                               AluOpType.mult, AluOpType.add)
        # td = (Vvec reshaped 3x3) @ v  (into rt12[9:12])
        prod = small.tile([1, 9], F32, tag="prod")
        V.tensor_mul(prod[:, :].rearrange("p (a c) -> p a c", a=3),
                     vv[:, :].rearrange("p (a c) -> p a c", a=3),
                     v3.unsqueeze(1).broadcast_to([1, 3, 3]))
        td = rt12[:, 9:12]
        pv = prod[:, :].rearrange("p (a c) -> p a c", a=3)
        tdv = td.rearrange("p (a c) -> p a c", a=3, c=1)
        V.tensor_add(tdv, pv[:, :, 0:1], pv[:, :, 1:2])
        V.tensor_add(tdv, tdv, pv[:, :, 2:3])

        # reshape [1,12] -> RdT|td [3,4]: transpose to a column, then
        # selector matmul Rt[k,m] = rtcol[3m+k] (no partition-reshape DMA)
        rtcol_ps = pps.tile([128, 512], F32, tag="sm")
        nc.tensor.transpose(rtcol_ps[0:12, 0:1], rt12[:, :], ident[0:1, 0:1])
        rtcol = small.tile([12, 1], F32, tag="rtcol")
        act(rtcol[:, :], rtcol_ps[0:12, 0:1], AF.Copy)
        rtmask = small.tile([12, 4], F32, tag="rtmask")
        V.tensor_mul(rtmask[:, :], rtc[:, 3:7], rtcol[:, :].to_broadcast([12, 4]))
        Rt_ps = pps.tile([128, 512], F32, tag="sm")
        nc.tensor.matmul(Rt_ps[0:3, 0:4], rtc[:, 0:3], rtmask[:, :],
                         start=True, stop=True)
        Rt = small.tile([3, 4], F32, tag="Rt")
        act(Rt[:, :], Rt_ps[0:3, 0:4], AF.Copy)
        if d_dbg is not None and it == 0:
            dma(d_dbg["dbg_pose"].ap(), pose[:, :])
            dma(d_dbg["dbg_rt12"].ap(), rt12[:, :])
            dma(d_dbg["dbg_Rt"].ap(), Rt[:, :])

        # compose
        Rn_ps = pps.tile([128, 512], F32, tag="sm")
        nc.tensor.matmul(Rn_ps[0:3, 0:3], Rt[:, 0:3], Rcur[:, :], start=True, stop=True)
        RnT_ps = pps.tile([128, 512], F32, tag="sm")
        nc.tensor.matmul(RnT_ps[0:3, 0:3], Rcur[:, :], Rt[:, 0:3], start=True, stop=True)
        tn_ps = pps.tile([128, 512], F32, tag="sm")
        nc.tensor.matmul(tn_ps[0:3, 0:1], Rt[:, 0:3], tcur[:, :], start=True, stop=True)
        Rcur = persist.tile([3, 3], F32, tag=f"Rcur{it}")
        RcurT = persist.tile([3, 3], F32, tag=f"RcurT{it}")
        tcur = persist.tile([3, 1], F32, tag=f"tcur{it}")
        act(Rcur[:, :], Rn_ps[0:3, 0:3], AF.Copy)
        act(RcurT[:, :], RnT_ps[0:3, 0:3], AF.Copy)
        V.tensor_scalar(tcur[:, :], tn_ps[0:3, 0:1], Rt[:, 3:4], None, AluOpType.add)
        if d_dbg is not None and it == 0:
            dma(d_dbg["dbg_Rcur1"].ap(), Rcur[:, :])

    if d_dbg is not None:
        dma(d_dbg["dbg_f0"].ap(), f0[:, :])
        dma(d_dbg["dbg_negmind"].ap(), negmind[:, :])
        dma(d_dbg["dbg_rank"].ap(), rank[:, :])
        dma(d_dbg["dbg_kpT"].ap(), kpT[:, :])
        dma(d_dbg["dbg_J"].ap(), J[:, :])
        dma(d_dbg["dbg_M"].ap(), M[:, :])

    # ---------------- output ----------------
    tf_ps = pps.tile([128, 512], F32, tag="sm")
    nc.tensor.matmul(tf_ps[0:3, 0:1], RcurT[:, :], sm[:, :], start=True, stop=True)
    te_tm = small.tile([3, 1], F32, tag="te_tm")
    V.tensor_add(te_tm[:, :], tcur[:, :], tm[:, :])
    tfin = small.tile([3, 1], F32, tag="tfin")
    V.tensor_scalar(tfin[:, :], tf_ps[0:3, 0:1], -1.0, te_tm[:, :],
                    AluOpType.mult, op1=AluOpType.add)
    yT = persist.tile([3, N_PTS], F32, tag="yT")
    for j in range(N_CHUNK):
        ps = pp.tile([128, 512], F32, tag="mm")
        nc.tensor.matmul(ps[0:3, :], RcurT[:, :], sT[:, 512 * j : 512 * (j + 1)],
                         start=True, stop=True)
        V.tensor_scalar(yT[:, 512 * j : 512 * (j + 1)], ps[0:3, :], tfin[:, :], None,
                        AluOpType.add)
    dma(d_out.ap(), yT[:, :])
    ctx.close()


# ---------------------------------------------------------------- host glue
_PROGRAM_CACHE = {}


def _get_program(maxiter):
    if maxiter not in _PROGRAM_CACHE:
        _PROGRAM_CACHE[maxiter] = build_program(maxiter)
    return _PROGRAM_CACHE[maxiter]


def make_in_maps(inputs):
    template = np.ascontiguousarray(np.asarray(inputs["template"], np.float32))
    source = np.ascontiguousarray(np.asarray(inputs["source"], np.float32))
    B = template.shape[0]
    consts = host_constants()
    Ws = {f"W{i}": np.ascontiguousarray(np.asarray(inputs[f"W{i}"], np.float32))
          for i in range(1, 6)}
    bs = {f"b{i}": np.ascontiguousarray(
        np.asarray(inputs[f"b{i}"], np.float32).reshape(-1, 1))
        for i in range(1, 5)}
    b5 = np.ascontiguousarray(
        np.asarray(inputs["b5"], np.float32).reshape(8, 128).T)
    in_maps = []
    for core in range(B):
        m = {
            "tT": np.ascontiguousarray(template[core].T),
            "sT": np.ascontiguousarray(source[core].T),
            "b5": b5,
            "jrot": consts["jrot"],
            "jtr": consts["jtr"],
            "coef": consts["coef"],
            "iota512": consts["iota512"],
            "tri128": consts["tri128"],
            "ident": consts["ident"],
            "rowbase": consts["rowbase"],
            "i9sgn": consts["i9sgn"],
            "gjc": consts["gjc"],
            "rtc": consts["rtc"],
        }
        m.update(Ws)
        m.update(bs)
        in_maps.append(m)
    return in_maps


def kernel(**inputs):
    from concourse.bass_utils import run_bass_kernel_spmd

    maxiter = int(np.asarray(inputs["maxiter"]))
    nc = _get_program(maxiter)
    if not getattr(nc, "_drains_split", False):
        split_multiwait_drains(nc)
        nc._drains_split = True
    in_maps = make_in_maps(inputs)
    B = len(in_maps)
    res = run_bass_kernel_spmd(nc, in_maps, core_ids=list(range(B)))
    out = np.stack([res.results[c]["yT"].T for c in range(B)]).astype(np.float32)
    return out


if __name__ == "__main__":
    nc = build_program(10)
    n = sum(len(b.instructions) for f in nc.m.functions for b in f.blocks)
    print(f"program built: {n} instructions")
